# revision 17
# baseline (speedup 1.0000x reference)
"""Trainium2 Bass kernel for a small dense transformer block.

Model (per reference):
  x : [B, T, D]  B=16, T=2048, D=40, H=4 heads, hs=10
  ln1 -> per-head q/k/v -> scores = k @ q^T (softmax over q index) -> @ Wp
  residual (on ln1(x)) -> ln2 -> FFN(relu) -> residual

Sharding: data-parallel over batch, 2 batches per core across 8 cores.

v2 design notes:
  - All matmul moving operands are bf16 (fp32/f32r stream at ~2-4 cyc/col
    on HW; bf16 streams 1 col/cyc).
  - exp() of the TxT scores is the hard floor (ScalarE-only op, ~218us/core
    at 33.5M elements). Split it: ACT does head-pair 0 natively; DVE does
    head-pair 1 with a one-op Schraudolph (int16(A*s+B) bitcast to bf16) on
    most j-steps. Softmax normalization cancels most of the sawtooth error
    (validated: full-block rel_l2 ~5e-3 vs 2e-2 budget).
  - Weights are packed host-side (numpy) into bf16 matmul layouts.
  - Softmax denominator rides the PV matmul via a ones-row in xnT and
    ones-entries in the packed Wv; 1/Z via reciprocal_approx_fast.
  - LN2 + FFN tail processed feature-major for chunk PAIRS at partition
    bases 0/64 so one DVE op covers two chunks; LN2 stats via tiny matmuls,
    rstd via ln/exp (stays in the exp table-set).
"""

import sys
from contextlib import ExitStack

for _p in ("/opt/trn_rl_repo",):
    if _p not in sys.path:
        sys.path.insert(0, _p)

import numpy as np

import concourse.bass as bass
import concourse.tile as tile
from concourse import mybir
from concourse.masks import make_identity

B_FULL = 16
N_CORES = 8
B_LOC = B_FULL // N_CORES
T = 2048
D = 40
H = 4
HS = 10
LN_EPS = 1e-5

F32 = mybir.dt.float32
BF16 = mybir.dt.bfloat16
I16 = mybir.dt.int16
I32 = mybir.dt.int32
U16 = mybir.dt.uint16
AF = mybir.ActivationFunctionType
OP = mybir.AluOpType

L2E = float(np.log2(np.e))
SCH_A = 128.0 * L2E
SCH_B = (127.0 - 0.0579) * 128.0
# pair1 exp goes to DVE when (j % DVE_MOD) < DVE_TAKE, else ACT
DVE_MOD = 8
DVE_TAKE = 5
USE_RECIP_APPROX = True
HEAT_N = 3


def build_kernel(b_loc=B_LOC, t_len=T, split_waits=True):
    nc = bass.Bass("TRN2", target_bir_lowering=False)

    x_d = nc.dram_tensor("x", [b_loc, t_len, D], F32, kind="ExternalInput")
    out_d = nc.dram_tensor("out", [b_loc, t_len, D], F32, kind="ExternalOutput")
    # packed weights (bf16 bits as uint16, packed host-side)
    wq_d = nc.dram_tensor("wq_p", [D, 128], U16, kind="ExternalInput")
    wk_d = nc.dram_tensor("wk_p", [D, 128], U16, kind="ExternalInput")
    wv_d = nc.dram_tensor("wv_p", [D + 1, 128], U16, kind="ExternalInput")
    wpp_d = nc.dram_tensor("wpp", [128, D], U16, kind="ExternalInput")
    sel_d = nc.dram_tensor("sel", [128, 128], U16, kind="ExternalInput")
    meanw_d = nc.dram_tensor("meanw", [64 + D, 1], U16, kind="ExternalInput")
    g2row_d = nc.dram_tensor("g2row", [65, D], U16, kind="ExternalInput")
    ng2_d = nc.dram_tensor("ng2row", [65, D], U16, kind="ExternalInput")
    id40_d = nc.dram_tensor("id40", [D, D], U16, kind="ExternalInput")
    w1_d = nc.dram_tensor("w1p", [64 + D, D], U16, kind="ExternalInput")
    w2_d = nc.dram_tensor("w2p", [64 + D, D], U16, kind="ExternalInput")
    ones_d = nc.dram_tensor("onesrow", [1, t_len], U16, kind="ExternalInput")
    g1c_d = nc.dram_tensor("g1c", [D, 1], F32, kind="ExternalInput")
    be1c_d = nc.dram_tensor("be1c", [D, 1], F32, kind="ExternalInput")
    bpc_d = nc.dram_tensor("bpc2", [128, 1], F32, kind="ExternalInput")
    b1c_d = nc.dram_tensor("b1c2", [128, 1], F32, kind="ExternalInput")
    b2c_d = nc.dram_tensor("b2c2", [128, 1], F32, kind="ExternalInput")
    be2c_d = nc.dram_tensor("be2c2", [128, 1], F32, kind="ExternalInput")

    n_tt = t_len // 128
    IC = 512 if t_len % 512 == 0 else t_len
    n_ic = t_len // IC
    NMA = min(512, t_len)

    with tile.TileContext(nc) as tc, ExitStack() as ctx:
        consts = ctx.enter_context(tc.tile_pool(name="consts", bufs=1))

        iden = consts.tile([128, 128], F32)
        make_identity(nc, iden)

        epsc = consts.tile([128, 1], F32)
        nc.vector.memset(epsc, LN_EPS)

        def load_u16(dram, shape, name):
            t_ = consts.tile(shape, U16, tag=name)
            nc.sync.dma_start(out=t_, in_=dram[:])
            return t_

        wq_s = load_u16(wq_d, [D, 128], "wq_s")
        wk_s = load_u16(wk_d, [D, 128], "wk_s")
        wv_s = load_u16(wv_d, [D + 1, 128], "wv_s")
        wpp_s = load_u16(wpp_d, [128, D], "wpp_s")
        sel_s = load_u16(sel_d, [128, 128], "sel_s")
        meanw_s = load_u16(meanw_d, [64 + D, 1], "meanw_s")
        g2row_s = load_u16(g2row_d, [65, D], "g2row_s")
        ng2_s = load_u16(ng2_d, [65, D], "ng2_s")
        id40_s = load_u16(id40_d, [D, D], "id40_s")
        w1_s = load_u16(w1_d, [64 + D, D], "w1_s")
        w2_s = load_u16(w2_d, [64 + D, D], "w2_s")

        wq_t = wq_s[:].bitcast(BF16)
        wk_t = wk_s[:].bitcast(BF16)
        wv_t = wv_s[:].bitcast(BF16)
        wpp_t = wpp_s[:].bitcast(BF16)
        sel_t = sel_s[:].bitcast(BF16)
        meanw_t = meanw_s[:].bitcast(BF16)
        g2row_t = g2row_s[:].bitcast(BF16)
        ng2_t = ng2_s[:].bitcast(BF16)
        id40_t = id40_s[:].bitcast(BF16)
        w1_t = w1_s[:].bitcast(BF16)
        w2_t = w2_s[:].bitcast(BF16)

        def load_col(dram, n, name):
            t_ = consts.tile([n, 1], F32, tag=name)
            nc.sync.dma_start(out=t_, in_=dram[:])
            return t_

        g1c = load_col(g1c_d, D, "g1c")
        be1c = load_col(be1c_d, D, "be1c")
        bpc2 = load_col(bpc_d, 128, "bpc2")
        b1c2 = load_col(b1c_d, 128, "b1c2")
        b2c2 = load_col(b2c_d, 128, "b2c2")
        be2c2 = load_col(be2c_d, 128, "be2c2")

        # ---------------- per-batch persistent SBUF ----------------
        persist = ctx.enter_context(tc.tile_pool(name="persist", bufs=1))
        xnT = [persist.tile([D + 1, t_len], BF16, tag=f"xnT{b}", name=f"xnT{b}")
               for b in range(b_loc)]
        qT = [persist.tile([128, t_len], BF16, tag=f"qT{b}", name=f"qT{b}")
              for b in range(b_loc)]
        kT = [persist.tile([128, t_len], BF16, tag=f"kT{b}", name=f"kT{b}")
              for b in range(b_loc)]
        vA = [persist.tile([128, n_tt, 128], BF16, tag=f"vA{b}", name=f"vA{b}")
              for b in range(b_loc)]
        mv = [persist.tile([128, n_tt, 2], F32, tag=f"mv{b}", name=f"mv{b}")
              for b in range(b_loc)]
        rstd = [persist.tile([128, n_tt], F32, tag=f"rstd{b}", name=f"rstd{b}")
                for b in range(b_loc)]

        sbA = ctx.enter_context(tc.tile_pool(name="sbA", bufs=4))
        xtp = ctx.enter_context(tc.tile_pool(name="xtp", bufs=10))

        with (
            tc.tile_pool(name="spool", bufs=1, space="PSUM") as sp,
            tc.tile_pool(name="pvpool", bufs=1, space="PSUM") as pvp,
            tc.tile_pool(name="psC", bufs=1, space="PSUM") as pC,
            tc.tile_pool(name="heat", bufs=1, space="PSUM") as htp,
            tc.tile_pool(name="psA", bufs=1, space="PSUM") as pA,
            tc.tile_pool(name="epool", bufs=2) as ep,
            tc.tile_pool(name="sbC", bufs=2) as sC,
            tc.tile_pool(name="pvsb", bufs=3) as pvs,
            tc.tile_pool(name="outp", bufs=4) as op_,
        ):
            GRP = 8

            def heater(n):
                """tiny independent matmuls that keep the PE array streaming
                through semaphore stalls so the HAM clock-gate stays warm"""
                for _ in range(n):
                    hp_ = htp.tile([1, 64], F32, tag="h", name="heat")
                    nc.tensor.matmul(hp_, lhsT=sel_t[0:1, 0:1],
                                     rhs=sel_t[0:1, 0:64],
                                     start=True, stop=True)

            # ================= stage A slices =================
            def emit_stage_a_slices(b):
                slices = []

                def stats2(t0, b=b):
                    for t_i in range(t0, min(t0 + 2, n_tt)):
                        xt = xtp.tile([128, D], F32, tag="xt", name="xt")
                        nc.sync.dma_start(
                            out=xt, in_=x_d[b, t_i * 128 : (t_i + 1) * 128, :])
                        st6 = sbA.tile([128, 6], F32, tag="st6", name="st6")
                        nc.vector.bn_stats(out=st6, in_=xt)
                        nc.vector.bn_aggr(out=mv[b][:, t_i, :], in_=st6)
                        xt_hold[(b, t_i)] = xt

                def rstd_g(g0, b=b):
                    lnv = sbA.tile([128, GRP], F32, tag="lnv", name="lnv")
                    nc.scalar.activation(out=lnv, in_=mv[b][:, g0 : g0 + GRP, 1],
                                         func=AF.Ln, bias=epsc, scale=1.0)
                    nc.scalar.activation(out=rstd[b][:, g0 : g0 + GRP], in_=lnv,
                                         func=AF.Exp, bias=0.0, scale=-0.5)

                def apply2(t0, b=b):
                    for t_i in range(t0, min(t0 + 2, n_tt)):
                        xt = xt_hold.pop((b, t_i))
                        xn = sbA.tile([128, D], F32, tag="xn", name="xn")
                        nc.vector.tensor_scalar(
                            out=xn, in0=xt,
                            scalar1=mv[b][:, t_i, 0:1],
                            scalar2=rstd[b][:, t_i : t_i + 1],
                            op0=OP.subtract, op1=OP.mult)
                        tp = pA.tile([D, 128], F32, tag="a", name="tp")
                        nc.tensor.transpose(tp, xn, iden)
                        nc.vector.tensor_scalar(
                            out=xnT[b][0:D, t_i * 128 : (t_i + 1) * 128],
                            in0=tp, scalar1=g1c, scalar2=be1c,
                            op0=OP.mult, op1=OP.add)

                def ones_row(b=b):
                    nc.sync.dma_start(out=xnT[b][D : D + 1, :].bitcast(U16),
                                      in_=ones_d[:])

                def q_chunk(c, b=b):
                    sl = slice(c * NMA, (c + 1) * NMA)
                    qp = pA.tile([128, NMA], F32, tag="a", name="qp")
                    nc.tensor.matmul(qp, lhsT=wq_t, rhs=xnT[b][0:D, sl],
                                     start=True, stop=True)
                    nc.vector.tensor_copy(out=qT[b][:, sl], in_=qp)

                def k_chunk(c, b=b):
                    sl = slice(c * NMA, (c + 1) * NMA)
                    kp = pA.tile([128, NMA], F32, tag="a", name="kp")
                    nc.tensor.matmul(kp, lhsT=wk_t, rhs=xnT[b][0:D, sl],
                                     start=True, stop=True)
                    nc.vector.tensor_copy(out=kT[b][:, sl], in_=kp)

                def v_group(g0, b=b):
                    for t_i in range(g0, min(g0 + 2, n_tt)):
                        vp = pA.tile([128, 128], F32, tag="a", name="vp")
                        nc.tensor.matmul(
                            vp, lhsT=xnT[b][:, t_i * 128 : (t_i + 1) * 128],
                            rhs=wv_t, start=True, stop=True)
                        nc.vector.tensor_copy(out=vA[b][:, t_i, :], in_=vp)

                for g0 in range(0, n_tt, GRP):
                    for t0 in range(g0, g0 + GRP, 2):
                        slices.append(lambda t0=t0: stats2(t0))
                    slices.append(lambda g0=g0: rstd_g(g0))
                    for t0 in range(g0, g0 + GRP, 2):
                        slices.append(lambda t0=t0: apply2(t0))
                slices.append(ones_row)
                for c in range(t_len // NMA):
                    slices.append(lambda c=c: q_chunk(c))
                    slices.append(lambda c=c: k_chunk(c))
                for g0 in range(0, n_tt, 2):
                    slices.append(lambda g0=g0: v_group(g0))
                return slices

            xt_hold = {}

            # ================= stage C (per chunk pair) =================
            def make_stage_c(b, i0, i1, hold):
                st = {}
                slices = []

                def norm(cc, b=b):
                    pv_sb = hold[cc]
                    zps = pC.tile([128, IC], F32, tag="c", name="zps")
                    nc.tensor.matmul(zps, lhsT=sel_t, rhs=pv_sb,
                                     start=True, stop=True)
                    # 1/Z: bitwise-not seed + one Newton step (max err ~0.26%)
                    y0 = sC.tile([128, IC], F32, tag="y0", name="y0")
                    nc.vector.tensor_scalar(
                        out=y0[:].bitcast(I32), in0=zps[:].bitcast(I32),
                        scalar1=float(0x7EF311C4), scalar2=-1.0,
                        op0=OP.subtract, op1=OP.mult)
                    nt = sC.tile([128, IC], F32, tag="nt", name="nt")
                    nc.vector.tensor_mul(out=nt, in0=y0, in1=zps)
                    nc.vector.tensor_scalar(out=nt, in0=nt, scalar1=-1.0,
                                            scalar2=2.0, op0=OP.mult,
                                            op1=OP.add)
                    rbc = sC.tile([128, IC], F32, tag="rbc", name="rbc")
                    nc.gpsimd.tensor_mul(out=rbc, in0=nt, in1=y0)
                    on = sC.tile([128, IC], BF16, tag=f"on{cc}", name="on")
                    nc.gpsimd.tensor_mul(out=on, in0=pv_sb, in1=rbc)
                    st[f"on{cc}"] = on

                def yp_x1(b=b):
                    yp = pC.tile([128, IC], F32, tag="c", name="yp")
                    for cc, on, ig in ((0, st["on0"], i0), (1, st["on1"], i1)):
                        base = 64 * cc
                        nc.tensor.matmul(yp[base : base + D, :], lhsT=wpp_t,
                                         rhs=on, start=True, stop=False,
                                         skip_group_check=True,
                                         tile_position=(0, base))
                        nc.tensor.matmul(yp[base : base + D, :], lhsT=id40_t,
                                         rhs=xnT[b][0:D, ig : ig + IC],
                                         start=False, stop=True,
                                         skip_group_check=True,
                                         tile_position=(0, base))
                    x1 = sC.tile([128, IC], BF16, tag="x1", name="x1")
                    nc.vector.tensor_scalar(
                        out=x1[0:104], in0=yp[0:104], scalar1=bpc2[0:104],
                        scalar2=None, op0=OP.add)
                    st["x1"] = x1

                def sq_stats():
                    x1 = st["x1"]
                    sq = sC.tile([128, IC], BF16, tag="sq", name="sq")
                    nc.gpsimd.tensor_mul(out=sq[0:104], in0=x1[0:104],
                                         in1=x1[0:104])
                    mu_ps = pC.tile([128, IC], F32, tag="c", name="mu_ps")
                    m2_ps = pC.tile([128, IC], F32, tag="c", name="m2_ps")
                    nc.tensor.matmul(mu_ps[0:1], lhsT=meanw_t[0:D],
                                     rhs=x1[0:D],
                                     start=True, stop=True,
                                     skip_group_check=True,
                                     tile_position=(0, 0))
                    nc.tensor.matmul(m2_ps[0:1], lhsT=meanw_t[0:D],
                                     rhs=sq[0:D],
                                     start=True, stop=True,
                                     skip_group_check=True,
                                     tile_position=(0, 0))
                    nc.tensor.matmul(mu_ps[64:65], lhsT=meanw_t[64 : 64 + D],
                                     rhs=x1[64 : 64 + D],
                                     start=True, stop=True,
                                     skip_group_check=True,
                                     tile_position=(64, 64))
                    nc.tensor.matmul(m2_ps[64:65], lhsT=meanw_t[64 : 64 + D],
                                     rhs=sq[64 : 64 + D],
                                     start=True, stop=True,
                                     skip_group_check=True,
                                     tile_position=(64, 64))
                    st["mu_ps"], st["m2_ps"] = mu_ps, m2_ps

                def rows():
                    rsd = sC.tile([65, IC], BF16, tag="rsd", name="rsd")
                    murs = sC.tile([65, IC], BF16, tag="murs", name="murs")
                    st["rsd"], st["murs"] = rsd, murs
                    msq = sC.tile([65, IC], F32, tag="msq", name="msq")
                    mu_sb = sC.tile([65, IC], F32, tag="mu_sb", name="mu_sb")
                    for cc in range(2):
                        pp = slice(64 * cc, 64 * cc + 1)
                        m2 = st["m2_ps"][pp, :]
                        nc.vector.tensor_copy(out=mu_sb[pp],
                                              in_=st["mu_ps"][pp, :])
                        mu = mu_sb[pp]
                        nc.vector.tensor_mul(out=msq[pp], in0=mu, in1=mu)
                        # var = m2 - mu^2, in place
                        nc.vector.scalar_tensor_tensor(
                            out=m2, in0=msq[pp], scalar=-1.0, in1=m2,
                            op0=OP.mult, op1=OP.add)
                        # rstd2 = exp(-0.5*ln(var+eps))
                        nc.scalar.activation(out=m2, in_=m2, func=AF.Ln,
                                             bias=epsc[pp], scale=1.0)
                        nc.scalar.activation(out=rsd[pp], in_=m2,
                                             func=AF.Exp, bias=0.0,
                                             scale=-0.5)
                        nc.vector.tensor_mul(out=murs[pp], in0=mu,
                                             in1=rsd[pp])

                def x2_op():
                    rsd, murs = st["rsd"], st["murs"]
                    up = pC.tile([128, IC], F32, tag="c", name="up")
                    nc.tensor.matmul(up[0:D], lhsT=g2row_t[0:1],
                                     rhs=rsd[0:1, :],
                                     start=True, stop=True,
                                     skip_group_check=True,
                                     tile_position=(0, 0))
                    nc.tensor.matmul(up[64 : 64 + D], lhsT=g2row_t[64:65],
                                     rhs=rsd[64:65, :],
                                     start=True, stop=True,
                                     skip_group_check=True,
                                     tile_position=(64, 64))
                    tt_ = sC.tile([128, IC], F32, tag="tt", name="tt")
                    nc.vector.tensor_mul(out=tt_[0:104], in0=st["x1"][0:104],
                                         in1=up[0:104])
                    w0p = pC.tile([128, IC], F32, tag="c", name="w0p")
                    nc.tensor.matmul(w0p[0:D], lhsT=ng2_t[0:1],
                                     rhs=murs[0:1, :],
                                     start=True, stop=True,
                                     skip_group_check=True,
                                     tile_position=(0, 0))
                    nc.tensor.matmul(w0p[64 : 64 + D], lhsT=ng2_t[64:65],
                                     rhs=murs[64:65, :],
                                     start=True, stop=True,
                                     skip_group_check=True,
                                     tile_position=(64, 64))
                    x2 = sC.tile([128, IC], BF16, tag="x2", name="x2")
                    nc.vector.scalar_tensor_tensor(
                        out=x2[0:104], in0=w0p[0:104], scalar=be2c2[0:104],
                        in1=tt_[0:104], op0=OP.add, op1=OP.add)
                    st["x2"] = x2

                def ffn1():
                    x2 = st["x2"]
                    hp = pC.tile([128, IC], F32, tag="c", name="hp")
                    nc.tensor.matmul(hp[0:D], lhsT=w1_t[0:D], rhs=x2[0:D],
                                     start=True, stop=True,
                                     skip_group_check=True,
                                     tile_position=(0, 0))
                    nc.tensor.matmul(hp[64 : 64 + D], lhsT=w1_t[64 : 64 + D],
                                     rhs=x2[64 : 64 + D],
                                     start=True, stop=True,
                                     skip_group_check=True,
                                     tile_position=(64, 64))
                    hs = sC.tile([128, IC], BF16, tag="hs", name="hs")
                    nc.vector.tensor_scalar(
                        out=hs[0:104], in0=hp[0:104], scalar1=b1c2[0:104],
                        scalar2=0.0, op0=OP.add, op1=OP.max)
                    st["hs"] = hs

                def ffn2():
                    y2 = pC.tile([128, IC], F32, tag="c", name="y2")
                    nc.tensor.matmul(y2[0:D], lhsT=w2_t[0:D], rhs=st["hs"][0:D],
                                     start=True, stop=True,
                                     skip_group_check=True,
                                     tile_position=(0, 0))
                    nc.tensor.matmul(y2[64 : 64 + D], lhsT=w2_t[64 : 64 + D],
                                     rhs=st["hs"][64 : 64 + D],
                                     start=True, stop=True,
                                     skip_group_check=True,
                                     tile_position=(64, 64))
                    ob = sC.tile([128, IC], F32, tag="ob", name="ob")
                    nc.vector.scalar_tensor_tensor(
                        out=ob[0:104], in0=y2[0:104], scalar=b2c2[0:104],
                        in1=st["x2"][0:104], op0=OP.add, op1=OP.add)
                    st["ob"] = ob

                def out2(cc, tt0, b=b):
                    ob = st["ob"]
                    base = 64 * cc
                    ig = i0 if cc == 0 else i1
                    for tt in range(tt0, tt0 + 2):
                        src = ob[base : base + D,
                                 tt * 128 : (tt + 1) * 128]
                        otp = pC.tile([128, D], F32, tag="c", name="otp")
                        nc.tensor.transpose(
                            otp, src,
                            iden[base : base + D, base : base + D])
                        osb = op_.tile([128, D], F32, tag="osb", name="osb")
                        nc.vector.tensor_copy(out=osb, in_=otp)
                        t_glob = ig + tt * 128
                        nc.sync.dma_start(
                            out=out_d[b, t_glob : t_glob + 128, :], in_=osb)

                slices.append(lambda: norm(0))
                slices.append(lambda: norm(1))
                slices.append(yp_x1)
                slices.append(sq_stats)
                slices.append(rows)
                slices.append(x2_op)
                slices.append(ffn1)
                slices.append(ffn2)
                for cc in range(2):
                    for tt0 in range(0, IC // 128, 2):
                        slices.append(lambda cc=cc, tt0=tt0: out2(cc, tt0))
                return slices

            # ================= main choreography =================
            for f in emit_stage_a_slices(0):
                f()
            a_queue = []
            for b2 in range(1, b_loc):
                a_queue.extend(emit_stage_a_slices(b2))

            c_queue = []
            pvsb_hold = {}
            last_pv = [None]
            gstep = [0]

            for b in range(b_loc):
                if b > 0:
                    while a_queue:
                        a_queue.pop(0)()
                for ic in range(n_ic):
                    i0 = ic * IC
                    isl = slice(i0, i0 + IC)
                    pv = pvp.tile([128, IC], F32, tag="pv")

                    def emit_pv(j, es, b=b, pv=pv, ic=ic):
                        for h in range(H):
                            e = es[h // 2]
                            if e.dtype == I16:
                                rhs = e[:, h % 2, :].bitcast(BF16)
                            else:
                                rhs = e[:, h % 2, :]
                            nc.tensor.matmul(
                                pv[32 * h : 32 * h + 32, :],
                                lhsT=vA[b][:, j, 32 * h : 32 * h + 32],
                                rhs=rhs,
                                start=(j == 0), stop=(j == n_tt - 1),
                                skip_group_check=True,
                                tile_position=(0, 32 * h))
                        if j == n_tt - 1:
                            pv_sb = pvs.tile([128, IC], BF16, tag="pvsb",
                                             name="pv_sb")
                            nc.vector.tensor_copy(out=pv_sb, in_=pv)
                            pvsb_hold[ic % 2] = pv_sb
                            if ic % 2 == 1:
                                c_queue.extend(make_stage_c(
                                    b, (ic - 1) * IC, ic * IC,
                                    dict(pvsb_hold)))

                    for j in range(n_tt):
                        jsl = slice(j * 128, (j + 1) * 128)
                        heater(HEAT_N)
                        s0 = sp.tile([128, 2, 512], F32, tag="s0", name="s0")
                        s1 = sp.tile([128, 2, 512], F32, tag="s1", name="s1")
                        for pair, s in ((0, s0), (1, s1)):
                            for k in range(2):
                                h = 2 * pair + k
                                hp = slice(32 * h, 32 * h + HS)
                                nc.tensor.matmul(
                                    s[:, k, 0:IC],
                                    lhsT=qT[b][hp, jsl],
                                    rhs=kT[b][hp, isl],
                                    start=True, stop=True,
                                    tile_position=(32 * h, 0))
                        e0 = ep.tile([128, 2, 512], BF16, tag="e0", name="e0")
                        nc.scalar.activation(out=e0[:, :, 0:IC],
                                             in_=s0[:, :, 0:IC], func=AF.Exp)
                        if (j % DVE_MOD) < DVE_TAKE:
                            e1 = ep.tile([128, 2, 512], I16, tag="e1",
                                         name="e1")
                            nc.vector.tensor_scalar(
                                out=e1[:, :, 0:IC], in0=s1[:, :, 0:IC],
                                scalar1=SCH_A, scalar2=SCH_B,
                                op0=OP.mult, op1=OP.add)
                        else:
                            e1 = ep.tile([128, 2, 512], BF16, tag="e1b",
                                         name="e1b")
                            nc.scalar.activation(out=e1[:, :, 0:IC],
                                                 in_=s1[:, :, 0:IC],
                                                 func=AF.Exp)
                        if last_pv[0] is not None:
                            heater(HEAT_N)
                            last_pv[0]()
                        last_pv[0] = (lambda j=j, es=(e0, e1), f=emit_pv:
                                      f(j, es))
                        if c_queue:
                            c_queue.pop(0)()
                        elif gstep[0] % 2 == 0 and a_queue:
                            a_queue.pop(0)()
                        gstep[0] += 1
            last_pv[0]()
            while c_queue:
                c_queue.pop(0)()

    if split_waits:
        _split_multiwaits(nc)
    return nc


def _split_multiwaits(nc):
    """walrus codegen in this container encodes a limited number of sem
    waits per instruction (1 for Drain, 2 for compute ops); spill extras
    onto preceding NOPs on the same engine. DMA copies are left alone —
    their waits ride in the DGE descriptor."""
    for func in nc.m.functions:
        for bb in func.blocks:
            insts = list(bb.instructions)
            out, changed = [], False
            for ins in insts:
                si = ins.sync_info
                maxw = 1
                if (maxw is not None and si is not None and si.on_wait
                        and len(si.on_wait) > maxw):
                    waits = list(si.on_wait)
                    for k, w in enumerate(waits[:-maxw]):
                        nop = mybir.InstNoOp(
                            name=f"{ins.name}-wsplit{k}",
                            sync_info=mybir.SyncInfo(on_wait=[w], on_update=[]),
                            bass_nofuse=True, engine=ins.engine)
                        try:
                            nc.register_instruction(nop, overwrite=True)
                        except Exception:
                            pass
                        out.append(nop)
                    si.on_wait = waits[-maxw:]
                    changed = True
                out.append(ins)
            if changed:
                bb.instructions = out


def _bfbits(a):
    u = np.ascontiguousarray(np.asarray(a, np.float32)).view(np.uint32)
    r = ((u >> 16) & 1) + 0x7FFF
    return ((u + r) >> 16).astype(np.uint16)


def make_weight_arrays(inputs):
    Wq = np.asarray(inputs["Wq"], np.float32)
    Wk = np.asarray(inputs["Wk"], np.float32)
    Wv = np.asarray(inputs["Wv"], np.float32)
    Wp = np.asarray(inputs["Wp"], np.float32)
    bp = np.asarray(inputs["bp"], np.float32)
    W1 = np.asarray(inputs["W1"], np.float32)
    b1 = np.asarray(inputs["b1"], np.float32)
    W2 = np.asarray(inputs["W2"], np.float32)
    b2 = np.asarray(inputs["b2"], np.float32)
    g1 = np.asarray(inputs["g1"], np.float32)
    be1 = np.asarray(inputs["be1"], np.float32)
    g2 = np.asarray(inputs["g2"], np.float32)
    be2 = np.asarray(inputs["be2"], np.float32)

    wq_p = np.zeros((D, 128), np.float32)
    wk_p = np.zeros((D, 128), np.float32)
    wv_p = np.zeros((D + 1, 128), np.float32)
    for h in range(H):
        wq_p[:, 32 * h : 32 * h + HS] = Wq[h]
        wk_p[:, 32 * h : 32 * h + HS] = Wk[h]
        wv_p[0:D, 32 * h : 32 * h + HS] = Wv[h]
        wv_p[D, 32 * h + HS] = 1.0
    wpp = np.zeros((128, D), np.float32)
    for h in range(H):
        wpp[32 * h : 32 * h + HS, :] = Wp[HS * h : HS * h + HS, :]
    sel = np.zeros((128, 128), np.float32)
    for h in range(H):
        sel[32 * h + HS, 32 * h : 32 * h + 32] = 1.0
    meanw = np.zeros((64 + D, 1), np.float32)
    meanw[0:D] = 1.0 / D
    meanw[64 : 64 + D] = 1.0 / D
    g2row = np.zeros((65, D), np.float32)
    g2row[0] = g2
    g2row[64] = g2
    ng2row = np.zeros((65, D), np.float32)
    ng2row[0] = -g2
    ng2row[64] = -g2
    id40 = np.eye(D, dtype=np.float32)
    onesrow = np.ones((1, T), np.float32)
    w1p = np.zeros((64 + D, D), np.float32)
    w1p[0:D] = W1
    w1p[64 : 64 + D] = W1
    w2p = np.zeros((64 + D, D), np.float32)
    w2p[0:D] = W2
    w2p[64 : 64 + D] = W2
    col2 = np.zeros((128, 1), np.float32)

    def c2(v):
        a = col2.copy()
        a[0:D, 0] = v
        a[64 : 64 + D, 0] = v
        return a

    return {
        "wq_p": _bfbits(wq_p), "wk_p": _bfbits(wk_p), "wv_p": _bfbits(wv_p),
        "wpp": _bfbits(wpp), "sel": _bfbits(sel), "meanw": _bfbits(meanw),
        "g2row": _bfbits(g2row), "ng2row": _bfbits(ng2row),
        "id40": _bfbits(id40), "onesrow": _bfbits(onesrow),
        "w1p": _bfbits(w1p), "w2p": _bfbits(w2p),
        "g1c": np.ascontiguousarray(g1.reshape(D, 1)),
        "be1c": np.ascontiguousarray(be1.reshape(D, 1)),
        "bpc2": c2(bp), "b1c2": c2(b1), "b2c2": c2(b2), "be2c2": c2(be2),
    }


def make_in_maps(inputs, n_cores=N_CORES):
    x = np.ascontiguousarray(np.asarray(inputs["x"], dtype=np.float32))
    b_loc = x.shape[0] // n_cores
    weights = make_weight_arrays(inputs)
    in_maps = []
    for c in range(n_cores):
        m = {"x": x[c * b_loc : (c + 1) * b_loc]}
        m.update(weights)
        in_maps.append(m)
    return in_maps


_NC_CACHE = {}


def kernel(**inputs):
    from concourse.bass_utils import run_bass_kernel_spmd

    x = np.asarray(inputs["x"])
    b_full = x.shape[0]
    n_cores = N_CORES
    b_loc = b_full // n_cores

    key = (b_loc, x.shape[1])
    if key not in _NC_CACHE:
        _NC_CACHE[key] = build_kernel(b_loc, x.shape[1])
    nc = _NC_CACHE[key]

    in_maps = make_in_maps(inputs, n_cores)
    res = run_bass_kernel_spmd(nc, in_maps, core_ids=list(range(n_cores)))
    out = np.concatenate([r["out"] for r in res.results], axis=0)
    return out


# revision 19
# speedup vs baseline: 1.2188x; 1.2188x over previous
"""Trainium2 Bass kernel for a small dense transformer block.

Model (per reference):
  x : [B, T, D]  B=16, T=2048, D=40, H=4 heads, hs=10
  ln1 -> per-head q/k/v -> scores = k @ q^T (softmax over q index) -> @ Wp
  residual (on ln1(x)) -> ln2 -> FFN(relu) -> residual

Sharding: data-parallel over batch, 2 batches per core across 8 cores.

v2 design notes:
  - All matmul moving operands are bf16 (fp32/f32r stream at ~2-4 cyc/col
    on HW; bf16 streams 1 col/cyc).
  - exp() of the TxT scores is the hard floor (ScalarE-only op, ~218us/core
    at 33.5M elements). Split it: ACT does head-pair 0 natively; DVE does
    head-pair 1 with a one-op Schraudolph (int16(A*s+B) bitcast to bf16) on
    most j-steps. Softmax normalization cancels most of the sawtooth error
    (validated: full-block rel_l2 ~5e-3 vs 2e-2 budget).
  - Weights are packed host-side (numpy) into bf16 matmul layouts.
  - Softmax denominator rides the PV matmul via a ones-row in xnT and
    ones-entries in the packed Wv; 1/Z via reciprocal_approx_fast.
  - LN2 + FFN tail processed feature-major for chunk PAIRS at partition
    bases 0/64 so one DVE op covers two chunks; LN2 stats via tiny matmuls,
    rstd via ln/exp (stays in the exp table-set).
"""

import sys
from contextlib import ExitStack

for _p in ("/opt/trn_rl_repo",):
    if _p not in sys.path:
        sys.path.insert(0, _p)

import numpy as np

import concourse.bass as bass
import concourse.bass_utils as _bass_utils
import concourse.tile as tile


from concourse import mybir
from concourse.masks import make_identity

B_FULL = 16
N_CORES = 8
B_LOC = B_FULL // N_CORES
T = 2048
D = 40
H = 4
HS = 10
LN_EPS = 1e-5

F32 = mybir.dt.float32
BF16 = mybir.dt.bfloat16
I16 = mybir.dt.int16
I32 = mybir.dt.int32
U16 = mybir.dt.uint16
AF = mybir.ActivationFunctionType
OP = mybir.AluOpType

L2E = float(np.log2(np.e))
SCH_A = 128.0 * L2E
SCH_B = (127.0 - 0.0579) * 128.0
# pair1 exp goes to DVE when (j % DVE_MOD) < DVE_TAKE, else ACT
DVE_MOD = 8
DVE_TAKE = 5
USE_RECIP_APPROX = True
HEAT_N = 0


def build_kernel(b_loc=B_LOC, t_len=T, split_waits=True):
    nc = bass.Bass("TRN2", target_bir_lowering=False)

    x_d = nc.dram_tensor("x", [b_loc, t_len, D], F32, kind="ExternalInput")
    out_d = nc.dram_tensor("out", [b_loc, t_len, D], F32, kind="ExternalOutput")
    # packed weights (bf16 bits as uint16, packed host-side)
    wq_d = nc.dram_tensor("wq_p", [D, 128], U16, kind="ExternalInput")
    wk_d = nc.dram_tensor("wk_p", [D, 128], U16, kind="ExternalInput")
    wv_d = nc.dram_tensor("wv_p", [D + 1, 128], U16, kind="ExternalInput")
    wpp_d = nc.dram_tensor("wpp", [128, D], U16, kind="ExternalInput")
    sel_d = nc.dram_tensor("sel", [128, 128], U16, kind="ExternalInput")
    meanw_d = nc.dram_tensor("meanw", [64 + D, 1], U16, kind="ExternalInput")
    g2row_d = nc.dram_tensor("g2row", [65, D], U16, kind="ExternalInput")
    ng2_d = nc.dram_tensor("ng2row", [65, D], U16, kind="ExternalInput")
    id40_d = nc.dram_tensor("id40", [D, D], U16, kind="ExternalInput")
    w1_d = nc.dram_tensor("w1p", [64 + D, D], U16, kind="ExternalInput")
    w2_d = nc.dram_tensor("w2p", [64 + D, D], U16, kind="ExternalInput")
    ones_d = nc.dram_tensor("onesrow", [1, t_len], U16, kind="ExternalInput")
    g1c_d = nc.dram_tensor("g1c", [D, 1], F32, kind="ExternalInput")
    be1c_d = nc.dram_tensor("be1c", [D, 1], F32, kind="ExternalInput")
    bpc_d = nc.dram_tensor("bpc2", [128, 1], F32, kind="ExternalInput")
    b1c_d = nc.dram_tensor("b1c2", [128, 1], F32, kind="ExternalInput")
    b2c_d = nc.dram_tensor("b2c2", [128, 1], F32, kind="ExternalInput")
    be2c_d = nc.dram_tensor("be2c2", [128, 1], F32, kind="ExternalInput")

    n_tt = t_len // 128
    IC = 512 if t_len % 512 == 0 else t_len
    n_ic = t_len // IC
    NMA = min(512, t_len)

    with tile.TileContext(nc) as tc, ExitStack() as ctx:
        consts = ctx.enter_context(tc.tile_pool(name="consts", bufs=1))

        iden = consts.tile([128, 128], F32)
        make_identity(nc, iden)

        epsc = consts.tile([128, 1], F32)
        nc.vector.memset(epsc, LN_EPS)

        def load_u16(dram, shape, name):
            t_ = consts.tile(shape, U16, tag=name)
            nc.sync.dma_start(out=t_, in_=dram[:])
            return t_

        wq_s = load_u16(wq_d, [D, 128], "wq_s")
        wk_s = load_u16(wk_d, [D, 128], "wk_s")
        wv_s = load_u16(wv_d, [D + 1, 128], "wv_s")
        wpp_s = load_u16(wpp_d, [128, D], "wpp_s")
        sel_s = load_u16(sel_d, [128, 128], "sel_s")
        meanw_s = load_u16(meanw_d, [64 + D, 1], "meanw_s")
        g2row_s = load_u16(g2row_d, [65, D], "g2row_s")
        ng2_s = load_u16(ng2_d, [65, D], "ng2_s")
        id40_s = load_u16(id40_d, [D, D], "id40_s")
        w1_s = load_u16(w1_d, [64 + D, D], "w1_s")
        w2_s = load_u16(w2_d, [64 + D, D], "w2_s")

        wq_t = wq_s[:].bitcast(BF16)
        wk_t = wk_s[:].bitcast(BF16)
        wv_t = wv_s[:].bitcast(BF16)
        wpp_t = wpp_s[:].bitcast(BF16)
        sel_t = sel_s[:].bitcast(BF16)
        meanw_t = meanw_s[:].bitcast(BF16)
        g2row_t = g2row_s[:].bitcast(BF16)
        ng2_t = ng2_s[:].bitcast(BF16)
        id40_t = id40_s[:].bitcast(BF16)
        w1_t = w1_s[:].bitcast(BF16)
        w2_t = w2_s[:].bitcast(BF16)

        def load_col(dram, n, name):
            t_ = consts.tile([n, 1], F32, tag=name)
            nc.sync.dma_start(out=t_, in_=dram[:])
            return t_

        g1c = load_col(g1c_d, D, "g1c")
        be1c = load_col(be1c_d, D, "be1c")
        bpc2 = load_col(bpc_d, 128, "bpc2")
        b1c2 = load_col(b1c_d, 128, "b1c2")
        b2c2 = load_col(b2c_d, 128, "b2c2")
        be2c2 = load_col(be2c_d, 128, "be2c2")

        # ---------------- per-batch persistent SBUF ----------------
        persist = ctx.enter_context(tc.tile_pool(name="persist", bufs=1))
        xnT = [persist.tile([D + 1, t_len], BF16, tag=f"xnT{b}", name=f"xnT{b}")
               for b in range(b_loc)]
        qT = [persist.tile([128, t_len], BF16, tag=f"qT{b}", name=f"qT{b}")
              for b in range(b_loc)]
        kT = [persist.tile([128, t_len], BF16, tag=f"kT{b}", name=f"kT{b}")
              for b in range(b_loc)]
        vA = [persist.tile([128, n_tt, 128], BF16, tag=f"vA{b}", name=f"vA{b}")
              for b in range(b_loc)]
        mv = [persist.tile([128, n_tt, 2], F32, tag=f"mv{b}", name=f"mv{b}")
              for b in range(b_loc)]
        rstd = [persist.tile([128, n_tt], F32, tag=f"rstd{b}", name=f"rstd{b}")
                for b in range(b_loc)]

        sbA = ctx.enter_context(tc.tile_pool(name="sbA", bufs=4))
        xtp = ctx.enter_context(tc.tile_pool(name="xtp", bufs=10))

        with (
            tc.tile_pool(name="spool", bufs=1, space="PSUM") as sp,
            tc.tile_pool(name="pvpool", bufs=1, space="PSUM") as pvp,
            tc.tile_pool(name="psC", bufs=2, space="PSUM") as pC,
            tc.tile_pool(name="psA", bufs=1, space="PSUM") as pA,
            tc.tile_pool(name="epool", bufs=2) as ep,
            tc.tile_pool(name="sbC", bufs=2) as sC,
            tc.tile_pool(name="pvsb", bufs=3) as pvs,
            tc.tile_pool(name="outp", bufs=4) as op_,
        ):
            GRP = 8

            # ================= stage A slices =================
            def emit_stage_a_slices(b):
                slices = []

                def stats2(t0, b=b):
                    for t_i in range(t0, min(t0 + 2, n_tt)):
                        xt = xtp.tile([128, D], F32, tag="xt", name="xt")
                        nc.sync.dma_start(
                            out=xt, in_=x_d[b, t_i * 128 : (t_i + 1) * 128, :])
                        st6 = sbA.tile([128, 6], F32, tag="st6", name="st6")
                        nc.vector.bn_stats(out=st6, in_=xt)
                        nc.vector.bn_aggr(out=mv[b][:, t_i, :], in_=st6)
                        xt_hold[(b, t_i)] = xt

                def rstd_g(g0, b=b):
                    lnv = sbA.tile([128, GRP], F32, tag="lnv", name="lnv")
                    nc.scalar.activation(out=lnv, in_=mv[b][:, g0 : g0 + GRP, 1],
                                         func=AF.Ln, bias=epsc, scale=1.0)
                    nc.scalar.activation(out=rstd[b][:, g0 : g0 + GRP], in_=lnv,
                                         func=AF.Exp, bias=0.0, scale=-0.5)

                def apply2(t0, b=b):
                    for t_i in range(t0, min(t0 + 2, n_tt)):
                        xt = xt_hold.pop((b, t_i))
                        xn = sbA.tile([128, D], F32, tag="xn", name="xn")
                        nc.vector.tensor_scalar(
                            out=xn, in0=xt,
                            scalar1=mv[b][:, t_i, 0:1],
                            scalar2=rstd[b][:, t_i : t_i + 1],
                            op0=OP.subtract, op1=OP.mult)
                        tp = pA.tile([D, 128], F32, tag="a", name="tp")
                        nc.tensor.transpose(tp, xn, iden)
                        nc.vector.tensor_scalar(
                            out=xnT[b][0:D, t_i * 128 : (t_i + 1) * 128],
                            in0=tp, scalar1=g1c, scalar2=be1c,
                            op0=OP.mult, op1=OP.add)

                def ones_row(b=b):
                    nc.sync.dma_start(out=xnT[b][D : D + 1, :].bitcast(U16),
                                      in_=ones_d[:])

                def q_chunk(c, b=b):
                    sl = slice(c * NMA, (c + 1) * NMA)
                    qp = pA.tile([128, NMA], F32, tag="a", name="qp")
                    nc.tensor.matmul(qp, lhsT=wq_t, rhs=xnT[b][0:D, sl],
                                     start=True, stop=True)
                    nc.vector.tensor_copy(out=qT[b][:, sl], in_=qp)

                def k_chunk(c, b=b):
                    sl = slice(c * NMA, (c + 1) * NMA)
                    kp = pA.tile([128, NMA], F32, tag="a", name="kp")
                    nc.tensor.matmul(kp, lhsT=wk_t, rhs=xnT[b][0:D, sl],
                                     start=True, stop=True)
                    nc.vector.tensor_copy(out=kT[b][:, sl], in_=kp)

                def v_group(g0, b=b):
                    for t_i in range(g0, min(g0 + 2, n_tt)):
                        vp = pA.tile([128, 128], F32, tag="a", name="vp")
                        nc.tensor.matmul(
                            vp, lhsT=xnT[b][:, t_i * 128 : (t_i + 1) * 128],
                            rhs=wv_t, start=True, stop=True)
                        nc.vector.tensor_copy(out=vA[b][:, t_i, :], in_=vp)

                for g0 in range(0, n_tt, GRP):
                    for t0 in range(g0, g0 + GRP, 2):
                        slices.append(lambda t0=t0: stats2(t0))
                    slices.append(lambda g0=g0: rstd_g(g0))
                    for t0 in range(g0, g0 + GRP, 2):
                        slices.append(lambda t0=t0: apply2(t0))
                slices.append(ones_row)
                for c in range(t_len // NMA):
                    slices.append(lambda c=c: q_chunk(c))
                    slices.append(lambda c=c: k_chunk(c))
                for g0 in range(0, n_tt, 2):
                    slices.append(lambda g0=g0: v_group(g0))
                return slices

            xt_hold = {}

            # ================= stage C (per chunk pair) =================
            def make_stage_c(b, i0, i1, hold):
                st = {}
                slices = []

                def norm(cc, b=b):
                    pv_sb = hold[cc]
                    zps = pC.tile([128, IC], F32, tag="c", name="zps")
                    nc.tensor.matmul(zps, lhsT=sel_t, rhs=pv_sb,
                                     start=True, stop=True)
                    # 1/Z: bitwise-not seed + one Newton step (max err ~0.26%)
                    y0 = sC.tile([128, IC], F32, tag="y0", name="y0")
                    nc.vector.tensor_scalar(
                        out=y0[:].bitcast(I32), in0=zps[:].bitcast(I32),
                        scalar1=float(0x7EF311C4), scalar2=-1.0,
                        op0=OP.subtract, op1=OP.mult)
                    nt = sC.tile([128, IC], F32, tag="nt", name="nt")
                    nc.vector.tensor_mul(out=nt, in0=y0, in1=zps)
                    nc.vector.tensor_scalar(out=nt, in0=nt, scalar1=-1.0,
                                            scalar2=2.0, op0=OP.mult,
                                            op1=OP.add)
                    rbc = sC.tile([128, IC], F32, tag="rbc", name="rbc")
                    nc.gpsimd.tensor_mul(out=rbc, in0=nt, in1=y0)
                    on = sC.tile([128, IC], BF16, tag=f"on{cc}", name="on")
                    nc.gpsimd.tensor_mul(out=on, in0=pv_sb, in1=rbc)
                    st[f"on{cc}"] = on

                def yp_x1(b=b):
                    yp = pC.tile([128, IC], F32, tag="c", name="yp")
                    for cc, on, ig in ((0, st["on0"], i0), (1, st["on1"], i1)):
                        base = 64 * cc
                        nc.tensor.matmul(yp[base : base + D, :], lhsT=wpp_t,
                                         rhs=on, start=True, stop=False,
                                         skip_group_check=True,
                                         tile_position=(0, base))
                        nc.tensor.matmul(yp[base : base + D, :], lhsT=id40_t,
                                         rhs=xnT[b][0:D, ig : ig + IC],
                                         start=False, stop=True,
                                         skip_group_check=True,
                                         tile_position=(0, base))
                    x1 = sC.tile([128, IC], BF16, tag="x1", name="x1")
                    nc.vector.tensor_scalar(
                        out=x1[0:104], in0=yp[0:104], scalar1=bpc2[0:104],
                        scalar2=None, op0=OP.add)
                    st["x1"] = x1

                def sq_stats():
                    x1 = st["x1"]
                    sq = sC.tile([128, IC], BF16, tag="sq", name="sq")
                    nc.gpsimd.tensor_mul(out=sq[0:104], in0=x1[0:104],
                                         in1=x1[0:104])
                    mu_ps = pC.tile([128, IC], F32, tag="c", name="mu_ps")
                    m2_ps = pC.tile([128, IC], F32, tag="c", name="m2_ps")
                    nc.tensor.matmul(mu_ps[0:1], lhsT=meanw_t[0:D],
                                     rhs=x1[0:D],
                                     start=True, stop=True,
                                     skip_group_check=True,
                                     tile_position=(0, 0))
                    nc.tensor.matmul(m2_ps[0:1], lhsT=meanw_t[0:D],
                                     rhs=sq[0:D],
                                     start=True, stop=True,
                                     skip_group_check=True,
                                     tile_position=(0, 0))
                    nc.tensor.matmul(mu_ps[64:65], lhsT=meanw_t[64 : 64 + D],
                                     rhs=x1[64 : 64 + D],
                                     start=True, stop=True,
                                     skip_group_check=True,
                                     tile_position=(64, 64))
                    nc.tensor.matmul(m2_ps[64:65], lhsT=meanw_t[64 : 64 + D],
                                     rhs=sq[64 : 64 + D],
                                     start=True, stop=True,
                                     skip_group_check=True,
                                     tile_position=(64, 64))
                    st["mu_ps"], st["m2_ps"] = mu_ps, m2_ps

                def rows():
                    rsd = sC.tile([65, IC], BF16, tag="rsd", name="rsd")
                    murs = sC.tile([65, IC], BF16, tag="murs", name="murs")
                    st["rsd"], st["murs"] = rsd, murs
                    msq = sC.tile([65, IC], F32, tag="msq", name="msq")
                    mu_sb = sC.tile([65, IC], F32, tag="mu_sb", name="mu_sb")
                    for cc in range(2):
                        pp = slice(64 * cc, 64 * cc + 1)
                        m2 = st["m2_ps"][pp, :]
                        nc.vector.tensor_copy(out=mu_sb[pp],
                                              in_=st["mu_ps"][pp, :])
                        mu = mu_sb[pp]
                        nc.vector.tensor_mul(out=msq[pp], in0=mu, in1=mu)
                        # var = m2 - mu^2, in place
                        nc.vector.scalar_tensor_tensor(
                            out=m2, in0=msq[pp], scalar=-1.0, in1=m2,
                            op0=OP.mult, op1=OP.add)
                        # rstd2 = exp(-0.5*ln(var+eps))
                        nc.scalar.activation(out=m2, in_=m2, func=AF.Ln,
                                             bias=epsc[pp], scale=1.0)
                        nc.scalar.activation(out=rsd[pp], in_=m2,
                                             func=AF.Exp, bias=0.0,
                                             scale=-0.5)
                        nc.vector.tensor_mul(out=murs[pp], in0=mu,
                                             in1=rsd[pp])

                def x2_op():
                    rsd, murs = st["rsd"], st["murs"]
                    up = pC.tile([128, IC], F32, tag="c", name="up")
                    nc.tensor.matmul(up[0:D], lhsT=g2row_t[0:1],
                                     rhs=rsd[0:1, :],
                                     start=True, stop=True,
                                     skip_group_check=True,
                                     tile_position=(0, 0))
                    nc.tensor.matmul(up[64 : 64 + D], lhsT=g2row_t[64:65],
                                     rhs=rsd[64:65, :],
                                     start=True, stop=True,
                                     skip_group_check=True,
                                     tile_position=(64, 64))
                    tt_ = sC.tile([128, IC], F32, tag="tt", name="tt")
                    nc.vector.tensor_mul(out=tt_[0:104], in0=st["x1"][0:104],
                                         in1=up[0:104])
                    w0p = pC.tile([128, IC], F32, tag="c", name="w0p")
                    nc.tensor.matmul(w0p[0:D], lhsT=ng2_t[0:1],
                                     rhs=murs[0:1, :],
                                     start=True, stop=True,
                                     skip_group_check=True,
                                     tile_position=(0, 0))
                    nc.tensor.matmul(w0p[64 : 64 + D], lhsT=ng2_t[64:65],
                                     rhs=murs[64:65, :],
                                     start=True, stop=True,
                                     skip_group_check=True,
                                     tile_position=(64, 64))
                    x2 = sC.tile([128, IC], BF16, tag="x2", name="x2")
                    nc.vector.scalar_tensor_tensor(
                        out=x2[0:104], in0=w0p[0:104], scalar=be2c2[0:104],
                        in1=tt_[0:104], op0=OP.add, op1=OP.add)
                    st["x2"] = x2

                def ffn1():
                    x2 = st["x2"]
                    hp = pC.tile([128, IC], F32, tag="c", name="hp")
                    nc.tensor.matmul(hp[0:D], lhsT=w1_t[0:D], rhs=x2[0:D],
                                     start=True, stop=True,
                                     skip_group_check=True,
                                     tile_position=(0, 0))
                    nc.tensor.matmul(hp[64 : 64 + D], lhsT=w1_t[64 : 64 + D],
                                     rhs=x2[64 : 64 + D],
                                     start=True, stop=True,
                                     skip_group_check=True,
                                     tile_position=(64, 64))
                    hs = sC.tile([128, IC], BF16, tag="hs", name="hs")
                    nc.vector.tensor_scalar(
                        out=hs[0:104], in0=hp[0:104], scalar1=b1c2[0:104],
                        scalar2=0.0, op0=OP.add, op1=OP.max)
                    st["hs"] = hs

                def ffn2():
                    y2 = pC.tile([128, IC], F32, tag="c", name="y2")
                    nc.tensor.matmul(y2[0:D], lhsT=w2_t[0:D], rhs=st["hs"][0:D],
                                     start=True, stop=True,
                                     skip_group_check=True,
                                     tile_position=(0, 0))
                    nc.tensor.matmul(y2[64 : 64 + D], lhsT=w2_t[64 : 64 + D],
                                     rhs=st["hs"][64 : 64 + D],
                                     start=True, stop=True,
                                     skip_group_check=True,
                                     tile_position=(64, 64))
                    ob = sC.tile([128, IC], F32, tag="ob", name="ob")
                    nc.vector.scalar_tensor_tensor(
                        out=ob[0:104], in0=y2[0:104], scalar=b2c2[0:104],
                        in1=st["x2"][0:104], op0=OP.add, op1=OP.add)
                    st["ob"] = ob

                def out2(cc, tt0, b=b):
                    ob = st["ob"]
                    base = 64 * cc
                    ig = i0 if cc == 0 else i1
                    for tt in range(tt0, tt0 + 2):
                        src = ob[base : base + D,
                                 tt * 128 : (tt + 1) * 128]
                        otp = pC.tile([128, D], F32, tag="c", name="otp")
                        nc.tensor.transpose(
                            otp, src,
                            iden[base : base + D, base : base + D])
                        osb = op_.tile([128, D], F32, tag="osb", name="osb")
                        nc.vector.tensor_copy(out=osb, in_=otp)
                        t_glob = ig + tt * 128
                        nc.sync.dma_start(
                            out=out_d[b, t_glob : t_glob + 128, :], in_=osb)

                slices.append(lambda: norm(0))
                slices.append(lambda: norm(1))
                slices.append(yp_x1)
                slices.append(sq_stats)
                slices.append(rows)
                slices.append(x2_op)
                slices.append(ffn1)
                slices.append(ffn2)
                for cc in range(2):
                    for tt0 in range(0, IC // 128, 2):
                        slices.append(lambda cc=cc, tt0=tt0: out2(cc, tt0))
                return slices

            # ================= main choreography =================
            for f in emit_stage_a_slices(0):
                f()
            a_queue = []
            for b2 in range(1, b_loc):
                a_queue.extend(emit_stage_a_slices(b2))

            c_queue = []
            pvsb_hold = {}
            last_pv = [None]
            gstep = [0]

            for b in range(b_loc):
                if b > 0:
                    while a_queue:
                        a_queue.pop(0)()
                for ic in range(n_ic):
                    i0 = ic * IC
                    isl = slice(i0, i0 + IC)
                    pv = pvp.tile([128, IC], F32, tag="pv")

                    def emit_pv(j, es, b=b, pv=pv, ic=ic):
                        for h in range(H):
                            e = es[h // 2]
                            if e.dtype == I16:
                                rhs = e[:, h % 2, :].bitcast(BF16)
                            else:
                                rhs = e[:, h % 2, :]
                            nc.tensor.matmul(
                                pv[32 * h : 32 * h + 32, :],
                                lhsT=vA[b][:, j, 32 * h : 32 * h + 32],
                                rhs=rhs,
                                start=(j == 0), stop=(j == n_tt - 1),
                                skip_group_check=True,
                                tile_position=(0, 32 * h))
                        if j == n_tt - 1:
                            pv_sb = pvs.tile([128, IC], BF16, tag="pvsb",
                                             name="pv_sb")
                            nc.vector.tensor_copy(out=pv_sb, in_=pv)
                            pvsb_hold[ic % 2] = pv_sb
                            if ic % 2 == 1:
                                c_queue.extend(make_stage_c(
                                    b, (ic - 1) * IC, ic * IC,
                                    dict(pvsb_hold)))

                    for j in range(n_tt):
                        jsl = slice(j * 128, (j + 1) * 128)
                        s0 = sp.tile([128, 2, 512], F32, tag="s0", name="s0")
                        s1 = sp.tile([128, 2, 512], F32, tag="s1", name="s1")
                        for pair, s in ((0, s0), (1, s1)):
                            for k in range(2):
                                h = 2 * pair + k
                                hp = slice(32 * h, 32 * h + HS)
                                nc.tensor.matmul(
                                    s[:, k, 0:IC],
                                    lhsT=qT[b][hp, jsl],
                                    rhs=kT[b][hp, isl],
                                    start=True, stop=True,
                                    tile_position=(32 * h, 0))
                        e0 = ep.tile([128, 2, 512], BF16, tag="e0", name="e0")
                        nc.scalar.activation(out=e0[:, :, 0:IC],
                                             in_=s0[:, :, 0:IC], func=AF.Exp)
                        if (j % DVE_MOD) < DVE_TAKE:
                            e1 = ep.tile([128, 2, 512], I16, tag="e1",
                                         name="e1")
                            nc.vector.tensor_scalar(
                                out=e1[:, :, 0:IC], in0=s1[:, :, 0:IC],
                                scalar1=SCH_A, scalar2=SCH_B,
                                op0=OP.mult, op1=OP.add)
                        else:
                            e1 = ep.tile([128, 2, 512], BF16, tag="e1b",
                                         name="e1b")
                            nc.scalar.activation(out=e1[:, :, 0:IC],
                                                 in_=s1[:, :, 0:IC],
                                                 func=AF.Exp)
                        if last_pv[0] is not None:
                            last_pv[0]()
                        last_pv[0] = (lambda j=j, es=(e0, e1), f=emit_pv:
                                      f(j, es))
                        if c_queue:
                            c_queue.pop(0)()
                        elif gstep[0] % 2 == 0 and a_queue:
                            a_queue.pop(0)()
                        gstep[0] += 1
            last_pv[0]()
            while c_queue:
                c_queue.pop(0)()

    if split_waits:
        _split_multiwaits(nc)
    return nc


def _split_multiwaits(nc):
    """walrus codegen in this container encodes a limited number of sem
    waits per instruction (1 for Drain, 2 for compute ops); spill extras
    onto preceding NOPs on the same engine. DMA copies are left alone —
    their waits ride in the DGE descriptor."""
    for func in nc.m.functions:
        for bb in func.blocks:
            insts = list(bb.instructions)
            out, changed = [], False
            for ins in insts:
                si = ins.sync_info
                maxw = 1
                if (maxw is not None and si is not None and si.on_wait
                        and len(si.on_wait) > maxw):
                    waits = list(si.on_wait)
                    for k, w in enumerate(waits[:-maxw]):
                        nop = mybir.InstNoOp(
                            name=f"{ins.name}-wsplit{k}",
                            sync_info=mybir.SyncInfo(on_wait=[w], on_update=[]),
                            bass_nofuse=True, engine=ins.engine)
                        try:
                            nc.register_instruction(nop, overwrite=True)
                        except Exception:
                            pass
                        out.append(nop)
                    si.on_wait = waits[-maxw:]
                    changed = True
                out.append(ins)
            if changed:
                bb.instructions = out


def _bfbits(a):
    u = np.ascontiguousarray(np.asarray(a, np.float32)).view(np.uint32)
    r = ((u >> 16) & 1) + 0x7FFF
    return ((u + r) >> 16).astype(np.uint16)


def make_weight_arrays(inputs):
    Wq = np.asarray(inputs["Wq"], np.float32)
    Wk = np.asarray(inputs["Wk"], np.float32)
    Wv = np.asarray(inputs["Wv"], np.float32)
    Wp = np.asarray(inputs["Wp"], np.float32)
    bp = np.asarray(inputs["bp"], np.float32)
    W1 = np.asarray(inputs["W1"], np.float32)
    b1 = np.asarray(inputs["b1"], np.float32)
    W2 = np.asarray(inputs["W2"], np.float32)
    b2 = np.asarray(inputs["b2"], np.float32)
    g1 = np.asarray(inputs["g1"], np.float32)
    be1 = np.asarray(inputs["be1"], np.float32)
    g2 = np.asarray(inputs["g2"], np.float32)
    be2 = np.asarray(inputs["be2"], np.float32)

    wq_p = np.zeros((D, 128), np.float32)
    wk_p = np.zeros((D, 128), np.float32)
    wv_p = np.zeros((D + 1, 128), np.float32)
    for h in range(H):
        wq_p[:, 32 * h : 32 * h + HS] = Wq[h]
        wk_p[:, 32 * h : 32 * h + HS] = Wk[h]
        wv_p[0:D, 32 * h : 32 * h + HS] = Wv[h]
        wv_p[D, 32 * h + HS] = 1.0
    wpp = np.zeros((128, D), np.float32)
    for h in range(H):
        wpp[32 * h : 32 * h + HS, :] = Wp[HS * h : HS * h + HS, :]
    sel = np.zeros((128, 128), np.float32)
    for h in range(H):
        sel[32 * h + HS, 32 * h : 32 * h + 32] = 1.0
    meanw = np.zeros((64 + D, 1), np.float32)
    meanw[0:D] = 1.0 / D
    meanw[64 : 64 + D] = 1.0 / D
    g2row = np.zeros((65, D), np.float32)
    g2row[0] = g2
    g2row[64] = g2
    ng2row = np.zeros((65, D), np.float32)
    ng2row[0] = -g2
    ng2row[64] = -g2
    id40 = np.eye(D, dtype=np.float32)
    onesrow = np.ones((1, T), np.float32)
    w1p = np.zeros((64 + D, D), np.float32)
    w1p[0:D] = W1
    w1p[64 : 64 + D] = W1
    w2p = np.zeros((64 + D, D), np.float32)
    w2p[0:D] = W2
    w2p[64 : 64 + D] = W2
    col2 = np.zeros((128, 1), np.float32)

    def c2(v):
        a = col2.copy()
        a[0:D, 0] = v
        a[64 : 64 + D, 0] = v
        return a

    return {
        "wq_p": _bfbits(wq_p), "wk_p": _bfbits(wk_p), "wv_p": _bfbits(wv_p),
        "wpp": _bfbits(wpp), "sel": _bfbits(sel), "meanw": _bfbits(meanw),
        "g2row": _bfbits(g2row), "ng2row": _bfbits(ng2row),
        "id40": _bfbits(id40), "onesrow": _bfbits(onesrow),
        "w1p": _bfbits(w1p), "w2p": _bfbits(w2p),
        "g1c": np.ascontiguousarray(g1.reshape(D, 1)),
        "be1c": np.ascontiguousarray(be1.reshape(D, 1)),
        "bpc2": c2(bp), "b1c2": c2(b1), "b2c2": c2(b2), "be2c2": c2(be2),
    }


def make_in_maps(inputs, n_cores=N_CORES):
    x = np.ascontiguousarray(np.asarray(inputs["x"], dtype=np.float32))
    b_loc = x.shape[0] // n_cores
    weights = make_weight_arrays(inputs)
    in_maps = []
    for c in range(n_cores):
        m = {"x": x[c * b_loc : (c + 1) * b_loc]}
        m.update(weights)
        in_maps.append(m)
    return in_maps


_NC_CACHE = {}


def kernel(**inputs):
    from concourse.bass_utils import run_bass_kernel_spmd

    x = np.asarray(inputs["x"])
    b_full = x.shape[0]
    n_cores = N_CORES
    b_loc = b_full // n_cores

    key = (b_loc, x.shape[1])
    if key not in _NC_CACHE:
        _NC_CACHE[key] = build_kernel(b_loc, x.shape[1])
    nc = _NC_CACHE[key]

    in_maps = make_in_maps(inputs, n_cores)
    res = run_bass_kernel_spmd(nc, in_maps, core_ids=list(range(n_cores)))
    out = np.concatenate([r["out"] for r in res.results], axis=0)
    return out


# revision 20
# speedup vs baseline: 1.4758x; 1.2109x over previous
"""Trainium2 Bass kernel for a small dense transformer block.

Model (per reference):
  x : [B, T, D]  B=16, T=2048, D=40, H=4 heads, hs=10
  ln1 -> per-head q/k/v -> scores = k @ q^T (softmax over q index) -> @ Wp
  residual (on ln1(x)) -> ln2 -> FFN(relu) -> residual

Sharding: data-parallel over batch, 2 batches per core across 8 cores.

v2 design notes:
  - All matmul moving operands are bf16 (fp32/f32r stream at ~2-4 cyc/col
    on HW; bf16 streams 1 col/cyc).
  - exp() of the TxT scores is the hard floor (ScalarE-only op, ~218us/core
    at 33.5M elements). Split it: ACT does head-pair 0 natively; DVE does
    head-pair 1 with a one-op Schraudolph (int16(A*s+B) bitcast to bf16) on
    most j-steps. Softmax normalization cancels most of the sawtooth error
    (validated: full-block rel_l2 ~5e-3 vs 2e-2 budget).
  - Weights are packed host-side (numpy) into bf16 matmul layouts.
  - Softmax denominator rides the PV matmul via a ones-row in xnT and
    ones-entries in the packed Wv; 1/Z via reciprocal_approx_fast.
  - LN2 + FFN tail processed feature-major for chunk PAIRS at partition
    bases 0/64 so one DVE op covers two chunks; LN2 stats via tiny matmuls,
    rstd via ln/exp (stays in the exp table-set).
"""

import sys
from contextlib import ExitStack

for _p in ("/opt/trn_rl_repo",):
    if _p not in sys.path:
        sys.path.insert(0, _p)

import numpy as np

import concourse.bass as bass
import concourse.bass_utils as _bass_utils
import concourse.tile as tile


from concourse import mybir
from concourse.masks import make_identity

B_FULL = 16
N_CORES = 8
B_LOC = B_FULL // N_CORES
T = 2048
D = 40
H = 4
HS = 10
LN_EPS = 1e-5

F32 = mybir.dt.float32
BF16 = mybir.dt.bfloat16
I16 = mybir.dt.int16
I32 = mybir.dt.int32
U16 = mybir.dt.uint16
AF = mybir.ActivationFunctionType
OP = mybir.AluOpType

L2E = float(np.log2(np.e))
SCH_A = 128.0 * L2E
SCH_B = (127.0 - 0.0579) * 128.0
# pair1 exp goes to DVE when (j % DVE_MOD) < DVE_TAKE, else ACT
DVE_MOD = 4
DVE_TAKE = 3
USE_RECIP_APPROX = True
HEAT_N = 0


def build_kernel(b_loc=B_LOC, t_len=T, split_waits=True):
    nc = bass.Bass("TRN2", target_bir_lowering=False)

    x_d = nc.dram_tensor("x", [b_loc, t_len, D], F32, kind="ExternalInput")
    out_d = nc.dram_tensor("out", [b_loc, t_len, D], F32, kind="ExternalOutput")
    # packed weights (bf16 bits as uint16, packed host-side)
    wq_d = nc.dram_tensor("wq_p", [D, 128], U16, kind="ExternalInput")
    wk_d = nc.dram_tensor("wk_p", [D, 128], U16, kind="ExternalInput")
    wv_d = nc.dram_tensor("wv_p", [D + 1, 128], U16, kind="ExternalInput")
    wpp_d = nc.dram_tensor("wpp", [128, D], U16, kind="ExternalInput")
    sel_d = nc.dram_tensor("sel", [128, 128], U16, kind="ExternalInput")
    meanw_d = nc.dram_tensor("meanw", [64 + D, 1], U16, kind="ExternalInput")
    g2row_d = nc.dram_tensor("g2row", [65, D], U16, kind="ExternalInput")
    ng2_d = nc.dram_tensor("ng2row", [65, D], U16, kind="ExternalInput")
    id40_d = nc.dram_tensor("id40", [D, D], U16, kind="ExternalInput")
    w1_d = nc.dram_tensor("w1p", [64 + D, D], U16, kind="ExternalInput")
    w2_d = nc.dram_tensor("w2p", [64 + D, D], U16, kind="ExternalInput")
    ones_d = nc.dram_tensor("onesrow", [1, t_len], U16, kind="ExternalInput")
    g1c_d = nc.dram_tensor("g1c", [D, 1], F32, kind="ExternalInput")
    be1c_d = nc.dram_tensor("be1c", [D, 1], F32, kind="ExternalInput")
    bpc_d = nc.dram_tensor("bpc2", [128, 1], F32, kind="ExternalInput")
    b1c_d = nc.dram_tensor("b1c2", [128, 1], F32, kind="ExternalInput")
    b2c_d = nc.dram_tensor("b2c2", [128, 1], F32, kind="ExternalInput")
    be2c_d = nc.dram_tensor("be2c2", [128, 1], F32, kind="ExternalInput")

    n_tt = t_len // 128
    IC = 512 if t_len % 512 == 0 else t_len
    n_ic = t_len // IC
    NMA = min(512, t_len)

    with tile.TileContext(nc) as tc, ExitStack() as ctx:
        consts = ctx.enter_context(tc.tile_pool(name="consts", bufs=1))

        iden = consts.tile([128, 128], F32)
        make_identity(nc, iden)

        epsc = consts.tile([128, 1], F32)
        nc.vector.memset(epsc, LN_EPS)

        def load_u16(dram, shape, name):
            t_ = consts.tile(shape, U16, tag=name)
            nc.sync.dma_start(out=t_, in_=dram[:])
            return t_

        wq_s = load_u16(wq_d, [D, 128], "wq_s")
        wk_s = load_u16(wk_d, [D, 128], "wk_s")
        wv_s = load_u16(wv_d, [D + 1, 128], "wv_s")
        wpp_s = load_u16(wpp_d, [128, D], "wpp_s")
        sel_s = load_u16(sel_d, [128, 128], "sel_s")
        meanw_s = load_u16(meanw_d, [64 + D, 1], "meanw_s")
        g2row_s = load_u16(g2row_d, [65, D], "g2row_s")
        ng2_s = load_u16(ng2_d, [65, D], "ng2_s")
        id40_s = load_u16(id40_d, [D, D], "id40_s")
        w1_s = load_u16(w1_d, [64 + D, D], "w1_s")
        w2_s = load_u16(w2_d, [64 + D, D], "w2_s")

        wq_t = wq_s[:].bitcast(BF16)
        wk_t = wk_s[:].bitcast(BF16)
        wv_t = wv_s[:].bitcast(BF16)
        wpp_t = wpp_s[:].bitcast(BF16)
        sel_t = sel_s[:].bitcast(BF16)
        meanw_t = meanw_s[:].bitcast(BF16)
        g2row_t = g2row_s[:].bitcast(BF16)
        ng2_t = ng2_s[:].bitcast(BF16)
        id40_t = id40_s[:].bitcast(BF16)
        w1_t = w1_s[:].bitcast(BF16)
        w2_t = w2_s[:].bitcast(BF16)

        def load_col(dram, n, name):
            t_ = consts.tile([n, 1], F32, tag=name)
            nc.sync.dma_start(out=t_, in_=dram[:])
            return t_

        g1c = load_col(g1c_d, D, "g1c")
        be1c = load_col(be1c_d, D, "be1c")
        bpc2 = load_col(bpc_d, 128, "bpc2")
        b1c2 = load_col(b1c_d, 128, "b1c2")
        b2c2 = load_col(b2c_d, 128, "b2c2")
        be2c2 = load_col(be2c_d, 128, "be2c2")

        # ---------------- per-batch persistent SBUF ----------------
        persist = ctx.enter_context(tc.tile_pool(name="persist", bufs=1))
        xnT = [persist.tile([D + 1, t_len], BF16, tag=f"xnT{b}", name=f"xnT{b}")
               for b in range(b_loc)]
        qT = [persist.tile([128, t_len], BF16, tag=f"qT{b}", name=f"qT{b}")
              for b in range(b_loc)]
        kT = [persist.tile([128, t_len], BF16, tag=f"kT{b}", name=f"kT{b}")
              for b in range(b_loc)]
        vA = [persist.tile([128, n_tt, 128], BF16, tag=f"vA{b}", name=f"vA{b}")
              for b in range(b_loc)]
        mv = [persist.tile([128, n_tt, 2], F32, tag=f"mv{b}", name=f"mv{b}")
              for b in range(b_loc)]
        rstd = [persist.tile([128, n_tt], F32, tag=f"rstd{b}", name=f"rstd{b}")
                for b in range(b_loc)]

        sbA = ctx.enter_context(tc.tile_pool(name="sbA", bufs=4))
        xtp = ctx.enter_context(tc.tile_pool(name="xtp", bufs=10))

        with (
            tc.tile_pool(name="spool", bufs=1, space="PSUM") as sp,
            tc.tile_pool(name="pvpool", bufs=1, space="PSUM") as pvp,
            tc.tile_pool(name="psC", bufs=2, space="PSUM") as pC,
            tc.tile_pool(name="psA", bufs=1, space="PSUM") as pA,
            tc.tile_pool(name="epool", bufs=3) as ep,
            tc.tile_pool(name="sbC", bufs=2) as sC,
            tc.tile_pool(name="pvsb", bufs=3) as pvs,
            tc.tile_pool(name="outp", bufs=4) as op_,
        ):
            GRP = 8

            # ================= stage A slices =================
            def emit_stage_a_slices(b):
                slices = []

                def stats2(t0, b=b):
                    for t_i in range(t0, min(t0 + 2, n_tt)):
                        xt = xtp.tile([128, D], F32, tag="xt", name="xt")
                        nc.sync.dma_start(
                            out=xt, in_=x_d[b, t_i * 128 : (t_i + 1) * 128, :])
                        st6 = sbA.tile([128, 6], F32, tag="st6", name="st6")
                        nc.vector.bn_stats(out=st6, in_=xt)
                        nc.vector.bn_aggr(out=mv[b][:, t_i, :], in_=st6)
                        xt_hold[(b, t_i)] = xt

                def rstd_g(g0, b=b):
                    lnv = sbA.tile([128, GRP], F32, tag="lnv", name="lnv")
                    nc.scalar.activation(out=lnv, in_=mv[b][:, g0 : g0 + GRP, 1],
                                         func=AF.Ln, bias=epsc, scale=1.0)
                    nc.scalar.activation(out=rstd[b][:, g0 : g0 + GRP], in_=lnv,
                                         func=AF.Exp, bias=0.0, scale=-0.5)

                def apply2(t0, b=b):
                    for t_i in range(t0, min(t0 + 2, n_tt)):
                        xt = xt_hold.pop((b, t_i))
                        xn = sbA.tile([128, D], F32, tag="xn", name="xn")
                        nc.vector.tensor_scalar(
                            out=xn, in0=xt,
                            scalar1=mv[b][:, t_i, 0:1],
                            scalar2=rstd[b][:, t_i : t_i + 1],
                            op0=OP.subtract, op1=OP.mult)
                        tp = pA.tile([D, 128], F32, tag="a", name="tp")
                        nc.tensor.transpose(tp, xn, iden)
                        nc.vector.tensor_scalar(
                            out=xnT[b][0:D, t_i * 128 : (t_i + 1) * 128],
                            in0=tp, scalar1=g1c, scalar2=be1c,
                            op0=OP.mult, op1=OP.add)

                def ones_row(b=b):
                    nc.sync.dma_start(out=xnT[b][D : D + 1, :].bitcast(U16),
                                      in_=ones_d[:])

                def q_chunk(c, b=b):
                    sl = slice(c * NMA, (c + 1) * NMA)
                    qp = pA.tile([128, NMA], F32, tag="a", name="qp")
                    nc.tensor.matmul(qp, lhsT=wq_t, rhs=xnT[b][0:D, sl],
                                     start=True, stop=True)
                    nc.vector.tensor_copy(out=qT[b][:, sl], in_=qp)

                def k_chunk(c, b=b):
                    sl = slice(c * NMA, (c + 1) * NMA)
                    kp = pA.tile([128, NMA], F32, tag="a", name="kp")
                    nc.tensor.matmul(kp, lhsT=wk_t, rhs=xnT[b][0:D, sl],
                                     start=True, stop=True)
                    nc.vector.tensor_copy(out=kT[b][:, sl], in_=kp)

                def v_group(g0, b=b):
                    for t_i in range(g0, min(g0 + 2, n_tt)):
                        vp = pA.tile([128, 128], F32, tag="a", name="vp")
                        nc.tensor.matmul(
                            vp, lhsT=xnT[b][:, t_i * 128 : (t_i + 1) * 128],
                            rhs=wv_t, start=True, stop=True)
                        nc.vector.tensor_copy(out=vA[b][:, t_i, :], in_=vp)

                for g0 in range(0, n_tt, GRP):
                    for t0 in range(g0, g0 + GRP, 2):
                        slices.append(lambda t0=t0: stats2(t0))
                    slices.append(lambda g0=g0: rstd_g(g0))
                    for t0 in range(g0, g0 + GRP, 2):
                        slices.append(lambda t0=t0: apply2(t0))
                slices.append(ones_row)
                for c in range(t_len // NMA):
                    slices.append(lambda c=c: q_chunk(c))
                    slices.append(lambda c=c: k_chunk(c))
                for g0 in range(0, n_tt, 2):
                    slices.append(lambda g0=g0: v_group(g0))
                return slices

            xt_hold = {}

            # ================= stage C (per chunk pair) =================
            def make_stage_c(b, i0, i1, hold):
                st = {}
                slices = []

                def norm(cc, b=b):
                    pv_sb = hold[cc]
                    zps = pC.tile([128, IC], F32, tag="c", name="zps")
                    nc.tensor.matmul(zps, lhsT=sel_t, rhs=pv_sb,
                                     start=True, stop=True)
                    # 1/Z: bitwise-not seed + one Newton step (max err ~0.26%)
                    y0 = sC.tile([128, IC], F32, tag="y0", name="y0")
                    nc.vector.tensor_scalar(
                        out=y0[:].bitcast(I32), in0=zps[:].bitcast(I32),
                        scalar1=float(0x7EF311C4), scalar2=-1.0,
                        op0=OP.subtract, op1=OP.mult)
                    nt = sC.tile([128, IC], F32, tag="nt", name="nt")
                    nc.vector.tensor_mul(out=nt, in0=y0, in1=zps)
                    nc.vector.tensor_scalar(out=nt, in0=nt, scalar1=-1.0,
                                            scalar2=2.0, op0=OP.mult,
                                            op1=OP.add)
                    rbc = sC.tile([128, IC], F32, tag="rbc", name="rbc")
                    nc.vector.tensor_mul(out=rbc, in0=nt, in1=y0)
                    on = sC.tile([128, IC], BF16, tag=f"on{cc}", name="on")
                    nc.vector.tensor_mul(out=on, in0=pv_sb, in1=rbc)
                    st[f"on{cc}"] = on

                def yp_x1(b=b):
                    yp = pC.tile([128, IC], F32, tag="c", name="yp")
                    for cc, on, ig in ((0, st["on0"], i0), (1, st["on1"], i1)):
                        base = 64 * cc
                        nc.tensor.matmul(yp[base : base + D, :], lhsT=wpp_t,
                                         rhs=on, start=True, stop=False,
                                         skip_group_check=True,
                                         tile_position=(0, base))
                        nc.tensor.matmul(yp[base : base + D, :], lhsT=id40_t,
                                         rhs=xnT[b][0:D, ig : ig + IC],
                                         start=False, stop=True,
                                         skip_group_check=True,
                                         tile_position=(0, base))
                    x1 = sC.tile([128, IC], BF16, tag="x1", name="x1")
                    nc.vector.tensor_scalar(
                        out=x1[0:104], in0=yp[0:104], scalar1=bpc2[0:104],
                        scalar2=None, op0=OP.add)
                    st["x1"] = x1

                def sq_stats():
                    x1 = st["x1"]
                    sq = sC.tile([128, IC], BF16, tag="sq", name="sq")
                    nc.vector.tensor_mul(out=sq[0:104], in0=x1[0:104],
                                          in1=x1[0:104])
                    mu_ps = pC.tile([128, IC], F32, tag="c", name="mu_ps")
                    m2_ps = pC.tile([128, IC], F32, tag="c", name="m2_ps")
                    nc.tensor.matmul(mu_ps[0:1], lhsT=meanw_t[0:D],
                                     rhs=x1[0:D],
                                     start=True, stop=True,
                                     skip_group_check=True,
                                     tile_position=(0, 0))
                    nc.tensor.matmul(m2_ps[0:1], lhsT=meanw_t[0:D],
                                     rhs=sq[0:D],
                                     start=True, stop=True,
                                     skip_group_check=True,
                                     tile_position=(0, 0))
                    nc.tensor.matmul(mu_ps[64:65], lhsT=meanw_t[64 : 64 + D],
                                     rhs=x1[64 : 64 + D],
                                     start=True, stop=True,
                                     skip_group_check=True,
                                     tile_position=(64, 64))
                    nc.tensor.matmul(m2_ps[64:65], lhsT=meanw_t[64 : 64 + D],
                                     rhs=sq[64 : 64 + D],
                                     start=True, stop=True,
                                     skip_group_check=True,
                                     tile_position=(64, 64))
                    st["mu_ps"], st["m2_ps"] = mu_ps, m2_ps

                def rows():
                    rsd = sC.tile([65, IC], BF16, tag="rsd", name="rsd")
                    murs = sC.tile([65, IC], BF16, tag="murs", name="murs")
                    st["rsd"], st["murs"] = rsd, murs
                    msq = sC.tile([65, IC], F32, tag="msq", name="msq")
                    mu_sb = sC.tile([65, IC], F32, tag="mu_sb", name="mu_sb")
                    for cc in range(2):
                        pp = slice(64 * cc, 64 * cc + 1)
                        m2 = st["m2_ps"][pp, :]
                        nc.vector.tensor_copy(out=mu_sb[pp],
                                              in_=st["mu_ps"][pp, :])
                        mu = mu_sb[pp]
                        nc.vector.tensor_mul(out=msq[pp], in0=mu, in1=mu)
                        # var = m2 - mu^2, in place
                        nc.vector.scalar_tensor_tensor(
                            out=m2, in0=msq[pp], scalar=-1.0, in1=m2,
                            op0=OP.mult, op1=OP.add)
                        # rstd2 = exp(-0.5*ln(var+eps))
                        nc.scalar.activation(out=m2, in_=m2, func=AF.Ln,
                                             bias=epsc[pp], scale=1.0)
                        nc.scalar.activation(out=rsd[pp], in_=m2,
                                             func=AF.Exp, bias=0.0,
                                             scale=-0.5)
                        nc.vector.tensor_mul(out=murs[pp], in0=mu,
                                             in1=rsd[pp])

                def x2_op():
                    rsd, murs = st["rsd"], st["murs"]
                    up = pC.tile([128, IC], F32, tag="c", name="up")
                    nc.tensor.matmul(up[0:D], lhsT=g2row_t[0:1],
                                     rhs=rsd[0:1, :],
                                     start=True, stop=True,
                                     skip_group_check=True,
                                     tile_position=(0, 0))
                    nc.tensor.matmul(up[64 : 64 + D], lhsT=g2row_t[64:65],
                                     rhs=rsd[64:65, :],
                                     start=True, stop=True,
                                     skip_group_check=True,
                                     tile_position=(64, 64))
                    tt_ = sC.tile([128, IC], F32, tag="tt", name="tt")
                    nc.vector.tensor_mul(out=tt_[0:104], in0=st["x1"][0:104],
                                         in1=up[0:104])
                    w0p = pC.tile([128, IC], F32, tag="c", name="w0p")
                    nc.tensor.matmul(w0p[0:D], lhsT=ng2_t[0:1],
                                     rhs=murs[0:1, :],
                                     start=True, stop=True,
                                     skip_group_check=True,
                                     tile_position=(0, 0))
                    nc.tensor.matmul(w0p[64 : 64 + D], lhsT=ng2_t[64:65],
                                     rhs=murs[64:65, :],
                                     start=True, stop=True,
                                     skip_group_check=True,
                                     tile_position=(64, 64))
                    x2 = sC.tile([128, IC], BF16, tag="x2", name="x2")
                    nc.vector.scalar_tensor_tensor(
                        out=x2[0:104], in0=w0p[0:104], scalar=be2c2[0:104],
                        in1=tt_[0:104], op0=OP.add, op1=OP.add)
                    st["x2"] = x2

                def ffn1():
                    x2 = st["x2"]
                    hp = pC.tile([128, IC], F32, tag="c", name="hp")
                    nc.tensor.matmul(hp[0:D], lhsT=w1_t[0:D], rhs=x2[0:D],
                                     start=True, stop=True,
                                     skip_group_check=True,
                                     tile_position=(0, 0))
                    nc.tensor.matmul(hp[64 : 64 + D], lhsT=w1_t[64 : 64 + D],
                                     rhs=x2[64 : 64 + D],
                                     start=True, stop=True,
                                     skip_group_check=True,
                                     tile_position=(64, 64))
                    hs = sC.tile([128, IC], BF16, tag="hs", name="hs")
                    nc.vector.tensor_scalar(
                        out=hs[0:104], in0=hp[0:104], scalar1=b1c2[0:104],
                        scalar2=0.0, op0=OP.add, op1=OP.max)
                    st["hs"] = hs

                def ffn2():
                    y2 = pC.tile([128, IC], F32, tag="c", name="y2")
                    nc.tensor.matmul(y2[0:D], lhsT=w2_t[0:D], rhs=st["hs"][0:D],
                                     start=True, stop=True,
                                     skip_group_check=True,
                                     tile_position=(0, 0))
                    nc.tensor.matmul(y2[64 : 64 + D], lhsT=w2_t[64 : 64 + D],
                                     rhs=st["hs"][64 : 64 + D],
                                     start=True, stop=True,
                                     skip_group_check=True,
                                     tile_position=(64, 64))
                    ob = sC.tile([128, IC], F32, tag="ob", name="ob")
                    nc.vector.scalar_tensor_tensor(
                        out=ob[0:104], in0=y2[0:104], scalar=b2c2[0:104],
                        in1=st["x2"][0:104], op0=OP.add, op1=OP.add)
                    st["ob"] = ob

                def out2(cc, tt0, b=b):
                    ob = st["ob"]
                    base = 64 * cc
                    ig = i0 if cc == 0 else i1
                    for tt in range(tt0, tt0 + 2):
                        src = ob[base : base + D,
                                 tt * 128 : (tt + 1) * 128]
                        otp = pC.tile([128, D], F32, tag="c", name="otp")
                        nc.tensor.transpose(
                            otp, src,
                            iden[base : base + D, base : base + D])
                        osb = op_.tile([128, D], F32, tag="osb", name="osb")
                        nc.vector.tensor_copy(out=osb, in_=otp)
                        t_glob = ig + tt * 128
                        nc.sync.dma_start(
                            out=out_d[b, t_glob : t_glob + 128, :], in_=osb)

                slices.append(lambda: norm(0))
                slices.append(lambda: norm(1))
                slices.append(yp_x1)
                slices.append(sq_stats)
                slices.append(rows)
                slices.append(x2_op)
                slices.append(ffn1)
                slices.append(ffn2)
                for cc in range(2):
                    for tt0 in range(0, IC // 128, 2):
                        slices.append(lambda cc=cc, tt0=tt0: out2(cc, tt0))
                return slices

            # ================= main choreography =================
            for f in emit_stage_a_slices(0):
                f()
            a_queue = []
            for b2 in range(1, b_loc):
                a_queue.extend(emit_stage_a_slices(b2))

            c_queue = []
            pvsb_hold = {}
            last_pv = [None]
            gstep = [0]

            for b in range(b_loc):
                if b > 0:
                    while a_queue:
                        a_queue.pop(0)()
                for ic in range(n_ic):
                    i0 = ic * IC
                    isl = slice(i0, i0 + IC)
                    pv = pvp.tile([128, IC], F32, tag="pv")

                    def emit_pv(j, es, b=b, pv=pv, ic=ic):
                        for h in range(H):
                            e = es[h // 2]
                            if e.dtype == I16:
                                rhs = e[:, h % 2, :].bitcast(BF16)
                            else:
                                rhs = e[:, h % 2, :]
                            nc.tensor.matmul(
                                pv[32 * h : 32 * h + 32, :],
                                lhsT=vA[b][:, j, 32 * h : 32 * h + 32],
                                rhs=rhs,
                                start=(j == 0), stop=(j == n_tt - 1),
                                skip_group_check=True,
                                tile_position=(0, 32 * h))
                        if j == n_tt - 1:
                            pv_sb = pvs.tile([128, IC], BF16, tag="pvsb",
                                             name="pv_sb")
                            nc.vector.tensor_copy(out=pv_sb, in_=pv)
                            pvsb_hold[ic % 2] = pv_sb
                            if ic % 2 == 1:
                                c_queue.extend(make_stage_c(
                                    b, (ic - 1) * IC, ic * IC,
                                    dict(pvsb_hold)))

                    for j in range(n_tt):
                        jsl = slice(j * 128, (j + 1) * 128)
                        s0 = sp.tile([128, 2, 512], F32, tag="s0", name="s0")
                        s1 = sp.tile([128, 2, 512], F32, tag="s1", name="s1")
                        for pair, s in ((0, s0), (1, s1)):
                            for k in range(2):
                                h = 2 * pair + k
                                hp = slice(32 * h, 32 * h + HS)
                                nc.tensor.matmul(
                                    s[:, k, 0:IC],
                                    lhsT=qT[b][hp, jsl],
                                    rhs=kT[b][hp, isl],
                                    start=True, stop=True,
                                    tile_position=(32 * h, 0))
                        e0 = ep.tile([128, 2, 512], BF16, tag="e0", name="e0")
                        nc.scalar.activation(out=e0[:, :, 0:IC],
                                             in_=s0[:, :, 0:IC], func=AF.Exp)
                        if (j % DVE_MOD) < DVE_TAKE:
                            e1 = ep.tile([128, 2, 512], I16, tag="e1",
                                         name="e1")
                            nc.vector.tensor_scalar(
                                out=e1[:, :, 0:IC], in0=s1[:, :, 0:IC],
                                scalar1=SCH_A, scalar2=SCH_B,
                                op0=OP.mult, op1=OP.add)
                        else:
                            e1 = ep.tile([128, 2, 512], BF16, tag="e1b",
                                         name="e1b")
                            nc.scalar.activation(out=e1[:, :, 0:IC],
                                                 in_=s1[:, :, 0:IC],
                                                 func=AF.Exp)
                        if last_pv[0] is not None:
                            last_pv[0]()
                        last_pv[0] = (lambda j=j, es=(e0, e1), f=emit_pv:
                                      f(j, es))
                        if c_queue:
                            c_queue.pop(0)()
                        elif gstep[0] % 2 == 0 and a_queue:
                            a_queue.pop(0)()
                        gstep[0] += 1
            last_pv[0]()
            while c_queue:
                c_queue.pop(0)()

    if split_waits:
        _split_multiwaits(nc)
    return nc


def _split_multiwaits(nc):
    """walrus codegen in this container encodes a limited number of sem
    waits per instruction (1 for Drain, 2 for compute ops); spill extras
    onto preceding NOPs on the same engine. DMA copies are left alone —
    their waits ride in the DGE descriptor."""
    for func in nc.m.functions:
        for bb in func.blocks:
            insts = list(bb.instructions)
            out, changed = [], False
            for ins in insts:
                si = ins.sync_info
                maxw = 1
                if (maxw is not None and si is not None and si.on_wait
                        and len(si.on_wait) > maxw):
                    waits = list(si.on_wait)
                    for k, w in enumerate(waits[:-maxw]):
                        nop = mybir.InstNoOp(
                            name=f"{ins.name}-wsplit{k}",
                            sync_info=mybir.SyncInfo(on_wait=[w], on_update=[]),
                            bass_nofuse=True, engine=ins.engine)
                        try:
                            nc.register_instruction(nop, overwrite=True)
                        except Exception:
                            pass
                        out.append(nop)
                    si.on_wait = waits[-maxw:]
                    changed = True
                out.append(ins)
            if changed:
                bb.instructions = out


def _bfbits(a):
    u = np.ascontiguousarray(np.asarray(a, np.float32)).view(np.uint32)
    r = ((u >> 16) & 1) + 0x7FFF
    return ((u + r) >> 16).astype(np.uint16)


def make_weight_arrays(inputs):
    Wq = np.asarray(inputs["Wq"], np.float32)
    Wk = np.asarray(inputs["Wk"], np.float32)
    Wv = np.asarray(inputs["Wv"], np.float32)
    Wp = np.asarray(inputs["Wp"], np.float32)
    bp = np.asarray(inputs["bp"], np.float32)
    W1 = np.asarray(inputs["W1"], np.float32)
    b1 = np.asarray(inputs["b1"], np.float32)
    W2 = np.asarray(inputs["W2"], np.float32)
    b2 = np.asarray(inputs["b2"], np.float32)
    g1 = np.asarray(inputs["g1"], np.float32)
    be1 = np.asarray(inputs["be1"], np.float32)
    g2 = np.asarray(inputs["g2"], np.float32)
    be2 = np.asarray(inputs["be2"], np.float32)

    wq_p = np.zeros((D, 128), np.float32)
    wk_p = np.zeros((D, 128), np.float32)
    wv_p = np.zeros((D + 1, 128), np.float32)
    for h in range(H):
        wq_p[:, 32 * h : 32 * h + HS] = Wq[h]
        wk_p[:, 32 * h : 32 * h + HS] = Wk[h]
        wv_p[0:D, 32 * h : 32 * h + HS] = Wv[h]
        wv_p[D, 32 * h + HS] = 1.0
    wpp = np.zeros((128, D), np.float32)
    for h in range(H):
        wpp[32 * h : 32 * h + HS, :] = Wp[HS * h : HS * h + HS, :]
    sel = np.zeros((128, 128), np.float32)
    for h in range(H):
        sel[32 * h + HS, 32 * h : 32 * h + 32] = 1.0
    meanw = np.zeros((64 + D, 1), np.float32)
    meanw[0:D] = 1.0 / D
    meanw[64 : 64 + D] = 1.0 / D
    g2row = np.zeros((65, D), np.float32)
    g2row[0] = g2
    g2row[64] = g2
    ng2row = np.zeros((65, D), np.float32)
    ng2row[0] = -g2
    ng2row[64] = -g2
    id40 = np.eye(D, dtype=np.float32)
    onesrow = np.ones((1, T), np.float32)
    w1p = np.zeros((64 + D, D), np.float32)
    w1p[0:D] = W1
    w1p[64 : 64 + D] = W1
    w2p = np.zeros((64 + D, D), np.float32)
    w2p[0:D] = W2
    w2p[64 : 64 + D] = W2
    col2 = np.zeros((128, 1), np.float32)

    def c2(v):
        a = col2.copy()
        a[0:D, 0] = v
        a[64 : 64 + D, 0] = v
        return a

    return {
        "wq_p": _bfbits(wq_p), "wk_p": _bfbits(wk_p), "wv_p": _bfbits(wv_p),
        "wpp": _bfbits(wpp), "sel": _bfbits(sel), "meanw": _bfbits(meanw),
        "g2row": _bfbits(g2row), "ng2row": _bfbits(ng2row),
        "id40": _bfbits(id40), "onesrow": _bfbits(onesrow),
        "w1p": _bfbits(w1p), "w2p": _bfbits(w2p),
        "g1c": np.ascontiguousarray(g1.reshape(D, 1)),
        "be1c": np.ascontiguousarray(be1.reshape(D, 1)),
        "bpc2": c2(bp), "b1c2": c2(b1), "b2c2": c2(b2), "be2c2": c2(be2),
    }


def make_in_maps(inputs, n_cores=N_CORES):
    x = np.ascontiguousarray(np.asarray(inputs["x"], dtype=np.float32))
    b_loc = x.shape[0] // n_cores
    weights = make_weight_arrays(inputs)
    in_maps = []
    for c in range(n_cores):
        m = {"x": x[c * b_loc : (c + 1) * b_loc]}
        m.update(weights)
        in_maps.append(m)
    return in_maps


_NC_CACHE = {}


def kernel(**inputs):
    from concourse.bass_utils import run_bass_kernel_spmd

    x = np.asarray(inputs["x"])
    b_full = x.shape[0]
    n_cores = N_CORES
    b_loc = b_full // n_cores

    key = (b_loc, x.shape[1])
    if key not in _NC_CACHE:
        _NC_CACHE[key] = build_kernel(b_loc, x.shape[1])
    nc = _NC_CACHE[key]

    in_maps = make_in_maps(inputs, n_cores)
    res = run_bass_kernel_spmd(nc, in_maps, core_ids=list(range(n_cores)))
    out = np.concatenate([r["out"] for r in res.results], axis=0)
    return out
